# revision 6
# baseline (speedup 1.0000x reference)
"""Trainium2 Bass kernel for a 5-layer MLP (512->256->128->64->32->512,
sigmoid on the first four layers) over batch 65536, data-parallel on 8 cores.

Contract: kernel(**inputs) takes the FULL unsharded inputs (np.ndarray, keyed
as in setup_inputs()) and returns the FULL [65536, 512] float32 output.

Strategy:
  - shard the batch across 8 NeuronCores (8192 rows each), replicate weights
  - activations kept transposed on-chip: h^T [features, batch_tile] so the
    feature dim lives on SBUF partitions -> bias+sigmoid are single ACT ops
  - fp16 I/O and matmul operands, fp32 PSUM accumulation (sim rel err ~4e-4)
  - host pre-transposes x to x^T [512, 8192] fp16 per shard so every DMA is
    contiguous 1KB-per-partition; y^T comes back fp16 and is transposed here
  - all weights packed into one [128, 1888] fp16 DMA and all biases into one
    [128, 9] fp32 DMA; a zero-dep warm-up sigmoid absorbs the ACT table load
"""

import numpy as np

import concourse.bass as bass
import concourse.mybir as mybir
import concourse.tile as tile
from concourse import bacc
from concourse.bass_utils import run_bass_kernel_spmd

N_CORES = 8
BATCH = 65536
B_C = BATCH // N_CORES  # 8192 rows per core
D_IN = 512
D_OUT = 512
NT = 512  # batch-tile size (matmul free dim)
N_TILES = B_C // NT  # 16

W_COLS = 1888  # 4*256 + 2*128 + 64 + 32 + 512
W2_OFF = 1024
W3_OFF = 1280
W4_OFF = 1344
W5_OFF = 1376

_f16 = mybir.dt.float16
_f32 = mybir.dt.float32


def _build_bass():
    # Bacc (not raw Bass): finalize() runs generate_event_semaphores, which
    # splits multi-sem waits into standalone EventSemaphore instructions --
    # TRN2 instructions can embed at most one sync wait.
    nc = bacc.Bacc(None)

    xt = nc.dram_tensor("xt", [D_IN, B_C], _f16, kind="ExternalInput")
    wp = nc.dram_tensor("wp", [128, W_COLS], _f16, kind="ExternalInput")
    bp = nc.dram_tensor("bp", [128, 9], _f32, kind="ExternalInput")
    yt = nc.dram_tensor("yt", [D_OUT, B_C], _f16, kind="ExternalOutput")

    sig = mybir.ActivationFunctionType.Sigmoid

    with tile.TileContext(nc) as tc:
        with (
            tc.tile_pool(name="consts", bufs=1) as consts,
            tc.tile_pool(name="xp", bufs=3) as xp,
            tc.tile_pool(name="hp", bufs=3) as hp,
            tc.tile_pool(name="yp", bufs=3) as yp,
            tc.tile_pool(name="ps1", bufs=3, space="PSUM") as ps1,
            tc.tile_pool(name="ps2", bufs=1, space="PSUM") as ps2,
            tc.tile_pool(name="ps3", bufs=1, space="PSUM") as ps3,
            tc.tile_pool(name="ps4", bufs=1, space="PSUM") as ps4,
            tc.tile_pool(name="ps5", bufs=2, space="PSUM") as ps5,
        ):
            # warm-up sigmoid with a single dependency: walrus attaches the
            # ACT table-load interlock to the first sigmoid in program order,
            # which costs a sync-wait slot on that instruction
            warm = consts.tile([1, 2], _f32)
            nc.vector.memset(warm[:], 0.0)
            nc.scalar.activation(warm[:, 0:1], warm[:, 0:1], sig, bias=warm[:, 1:2])

            ws = consts.tile([128, W_COLS], _f16)
            nc.sync.dma_start(ws[:], wp[:])
            bs = consts.tile([128, 9], _f32)
            nc.sync.dma_start(bs[:], bp[:])

            def w1s(k, m):
                return ws[:, k * 256 + m * 128 : k * 256 + (m + 1) * 128]

            def w2s(k):
                return ws[:, W2_OFF + k * 128 : W2_OFF + (k + 1) * 128]

            xt3 = xt[:].rearrange("(ko p) n -> p ko n", p=128)  # [128, 4, B_C]
            yt3 = yt[:].rearrange("(mo p) n -> p mo n", p=128)  # [128, 4, B_C]

            for t in range(N_TILES):
                tsl = bass.ts(t, NT)

                xk = xp.tile([128, 4, NT], _f16, tag="xk")
                nc.sync.dma_start(xk[:], xt3[:, :, tsl])

                # L1: 512 -> 256
                h1 = hp.tile([128, 2, NT], _f16, tag="h1")
                for m in range(2):
                    p1 = ps1.tile([128, NT], _f32, tag="p1")
                    for k in range(4):
                        nc.tensor.matmul(
                            p1[:],
                            w1s(k, m),
                            xk[:, k, :],
                            start=(k == 0),
                            stop=(k == 3),
                        )
                    nc.scalar.activation(h1[:, m, :], p1[:], sig, bias=bs[:, m : m + 1])

                # L2: 256 -> 128
                p2 = ps2.tile([128, NT], _f32, tag="p2")
                for k in range(2):
                    nc.tensor.matmul(
                        p2[:], w2s(k), h1[:, k, :], start=(k == 0), stop=(k == 1)
                    )
                h2 = hp.tile([128, NT], _f16, tag="h2")
                nc.scalar.activation(h2[:], p2[:], sig, bias=bs[:, 2:3])

                # L3: 128 -> 64
                p3 = ps3.tile([64, NT], _f32, tag="p3")
                nc.tensor.matmul(p3[:], ws[:, W3_OFF : W3_OFF + 64], h2[:], start=True, stop=True)
                h3 = hp.tile([64, NT], _f16, tag="h3")
                nc.scalar.activation(h3[:], p3[:], sig, bias=bs[:64, 3:4])

                # L4: 64 -> 32
                p4 = ps4.tile([32, NT], _f32, tag="p4")
                nc.tensor.matmul(p4[:], ws[:64, W4_OFF : W4_OFF + 32], h3[:], start=True, stop=True)
                h4 = hp.tile([32, NT], _f16, tag="h4")
                nc.scalar.activation(h4[:], p4[:], sig, bias=bs[:32, 4:5])

                # L5: 32 -> 512 (no activation; bias on VectorE)
                yts = yp.tile([128, 4, NT], _f16, tag="yts")
                for m in range(4):
                    p5 = ps5.tile([128, NT], _f32, tag="p5")
                    nc.tensor.matmul(
                        p5[:],
                        ws[:32, W5_OFF + m * 128 : W5_OFF + (m + 1) * 128],
                        h4[:],
                        start=True,
                        stop=True,
                    )
                    nc.vector.tensor_scalar_add(yts[:, m, :], p5[:], bs[:, 5 + m : 6 + m])

                nc.sync.dma_start(yt3[:, :, tsl], yts[:])

    nc.finalize()
    return nc


_NC_CACHE = None


def _get_nc():
    global _NC_CACHE
    if _NC_CACHE is None:
        _NC_CACHE = _build_bass()
    return _NC_CACHE


def _pack_consts(w1, b1, w2, b2, w3, b3, w4, b4, w5, b5):
    wp = np.zeros((128, W_COLS), dtype=np.float16)
    for k in range(4):
        wp[:, k * 256 : (k + 1) * 256] = w1.T[k * 128 : (k + 1) * 128, :]
    for k in range(2):
        wp[:, W2_OFF + k * 128 : W2_OFF + (k + 1) * 128] = w2.T[k * 128 : (k + 1) * 128, :]
    wp[:, W3_OFF : W3_OFF + 64] = w3.T
    wp[:64, W4_OFF : W4_OFF + 32] = w4.T
    wp[:32, W5_OFF : W5_OFF + 512] = w5.T

    bpk = np.zeros((128, 9), dtype=np.float32)
    bpk[:, 0] = b1[:128]
    bpk[:, 1] = b1[128:]
    bpk[:, 2] = b2
    bpk[:64, 3] = b3
    bpk[:32, 4] = b4
    for m in range(4):
        bpk[:, 5 + m] = b5[m * 128 : (m + 1) * 128]
    return np.ascontiguousarray(wp), np.ascontiguousarray(bpk)


def _make_in_maps(x, w1, b1, w2, b2, w3, b3, w4, b4, w5, b5):
    wp, bpk = _pack_consts(w1, b1, w2, b2, w3, b3, w4, b4, w5, b5)
    shared = {"wp": wp, "bp": bpk}
    in_maps = []
    for c in range(N_CORES):
        shard = x[c * B_C : (c + 1) * B_C]  # [B_C, 512]
        xtc = np.ascontiguousarray(shard.T.astype(np.float16))  # [512, B_C]
        in_maps.append({"xt": xtc, **shared})
    return in_maps


def _postprocess(x, results):
    y = np.empty((BATCH, D_OUT), dtype=np.float32)
    for c in range(N_CORES):
        y[c * B_C : (c + 1) * B_C] = results[c]["yt"].T.astype(np.float32)
    # reference: out[:, :in_size] = y, rest zero, in_size = count_nonzero(x[0])
    in_size = int(np.count_nonzero(x[0]))
    if in_size < D_OUT:
        y[:, in_size:] = 0.0
    return y


def run_traced(inputs, trace=False):
    """Run on 8 cores; returns (y_full, BassKernelResults)."""
    nc = _get_nc()
    in_maps = _make_in_maps(**inputs)
    res = run_bass_kernel_spmd(nc, in_maps, core_ids=list(range(N_CORES)), trace=trace)
    y = _postprocess(inputs["x"], res.results)
    return y, res


def kernel(**inputs) -> np.ndarray:
    y, _ = run_traced(inputs, trace=False)
    return y
